# revision 1
# baseline (speedup 1.0000x reference)
"""Batch Child-Sum TreeLSTM on 8 Trainium2 NeuronCores.

Strategy
--------
Data-parallel over batch: each of the 8 cores gets 16 of the 128 batch rows
and runs the full 64-step tree recurrence independently (no collectives).

Per core the recurrence is reformulated as an append-only *history*:
at step t we append (c_t, h_t, g_t = h_t @ Wfh.T) as column t of SBUF-resident
history tensors.  The reference's masked sums over the 64 node slots
(h_sum = sum_s cmask*H[s], fc = sum_s cmask*sigmoid(G[s]+fx)*C[s]) become
masked sums over the step-prefix tau < t with host-precomputed 0/1 masks
w_t[b,tau] ("tau is the latest write of its node before t AND that node is a
child of the node processed at step t").  The masks are exact, so the result
matches the reference up to fp rounding.

Layout ("B layout"): SBUF partition p = idx % 128 for every 512-dim vector
(hidden state h-space and gate j-space), free dims = (chunk = idx//128, b).
Histories are [128, tau, chunk*16b].  Nothing ever needs a per-step transpose:
h is born in matmul (k-on-partition) layout and is directly the rhs of the
next matmuls.  Masks are broadcast across partitions once per step with a
K=1 ones-matmul into PSUM.

The x-side gate projections for all 64 steps are one big GEMM done up front
(embedding rows are host-gathered in step order into XE^T since bfs/x are
host-visible index tensors; a bias row is folded in) and kept in SBUF.
"""

import numpy as np

import concourse.bass as bass
import concourse.bacc as bacc
import concourse.tile as tile
from concourse import mybir
from concourse.bass_utils import run_bass_kernel_spmd

F32 = mybir.dt.float32
BF16 = mybir.dt.bfloat16
AF = mybir.ActivationFunctionType
ALU = mybir.AluOpType

NCORES = 8
B, S, E, H, V, L = 128, 64, 300, 512, 100000, 5
BC = B // NCORES  # 16 batch rows per core
EK = 304          # padded E+bias rows (300 data + 1 ones + 3 zero)
KC = [128, 128, 48]  # K-chunk sizes for the 304-row x-side contraction

TRACE = False
LINEARIZE = False
LAST_RESULT = None

_prog_cache = {}


def _bcast_mid(ap, n):
    """[P, F] AP -> [P, n, F] AP with a step-0 broadcast middle dim."""
    return bass.AP(tensor=ap.tensor, offset=ap.offset,
                   ap=[ap.ap[0], [0, n], *ap.ap[1:]])


def _ap(ap, dims):
    """New AP over same tensor/offset with explicit free dims [[step,count],..]."""
    return bass.AP(tensor=ap.tensor, offset=ap.offset, ap=[ap.ap[0], *dims])


def _build_program(repeat=1):
    key = ("nc", repeat)
    if key in _prog_cache:
        return _prog_cache[key]

    nc = bacc.Bacc(None, target_bir_lowering=False, debug=False)

    xet_d = nc.declare_dram_parameter("xet", [EK, S * BC], F32, isOutput=False)
    wxt_d = nc.declare_dram_parameter("wxt", [EK, 3 * H], F32, isOutput=False)
    wht_d = nc.declare_dram_parameter("wht", [H, 3 * H], F32, isOutput=False)
    wgt_d = nc.declare_dram_parameter("wgt", [H, H], F32, isOutput=False)
    wout_d = nc.declare_dram_parameter("wout", [H, L], F32, isOutput=False)
    bout_d = nc.declare_dram_parameter("bout_rep", [BC, L], F32, isOutput=False)
    wmrow_d = nc.declare_dram_parameter("wmrow", [S, S * BC], F32, isOutput=False)
    out_d = nc.declare_dram_parameter("out", [BC, L], F32, isOutput=True)

    with tile.TileContext(nc, linearize=LINEARIZE) as tc:
        with (
            tc.tile_pool(name="singles", bufs=1) as sg,
        ):
            # ---------------- persistent state ----------------
            g_hist = sg.tile([128, S, 64], F32)   # G history (j-space)
            c_hist = sg.tile([128, S, 64], F32)   # C history
            ht = sg.tile([128, S, 64], F32)       # H^T history (k-layout)
            zx_all = sg.tile([128, S, 192], F32)  # x-side gate terms per step
            wht_s = sg.tile([128, 4, 3 * H], F32)
            wgt_s = sg.tile([128, 4, H], F32)
            wout_s = sg.tile([128, 4, L], F32)
            bout_s = sg.tile([BC, L], F32)
            ones_s = sg.tile([1, 128], F32)
            # ------------- startup loads (SWDGE: one sem each) -------------
            nc.gpsimd.dma_start(
                out=wht_s[:],
                in_=wht_d[:].rearrange("(k1 p) n -> p k1 n", p=128))
            nc.gpsimd.dma_start(
                out=wgt_s[:],
                in_=wgt_d[:].rearrange("(k1 p) n -> p k1 n", p=128))
            nc.gpsimd.dma_start(
                out=wout_s[:],
                in_=wout_d[:].rearrange("(k1 p) n -> p k1 n", p=128))
            nc.gpsimd.dma_start(out=bout_s[:], in_=bout_d[:])
            nc.vector.memset(ones_s[:], 1.0)

            # ---------------- phase 1: x-side GEMM ----------------
            with (
                tc.tile_pool(name="xw", bufs=1) as xw,
                tc.tile_pool(name="ph1p", bufs=2, space="PSUM") as ph1p,
            ):
                wxt_c = [xw.tile([128, 3 * H], F32, name=f"wxt{k}",
                                 tag=f"wxt{k}") for k in range(3)]
                xet_c = [xw.tile([128, S * BC], F32, name=f"xet{k}",
                                 tag=f"xet{k}") for k in range(3)]
                for k1 in range(3):
                    cnt = KC[k1]
                    nc.gpsimd.dma_start(out=wxt_c[k1][0:cnt, :],
                                        in_=wxt_d[k1 * 128:k1 * 128 + cnt, :])
                    nc.gpsimd.dma_start(out=xet_c[k1][0:cnt, :],
                                        in_=xet_d[k1 * 128:k1 * 128 + cnt, :])
                for g in range(3):
                    for c in range(4):
                        for nh in range(2):
                            zxp = ph1p.tile([128, 512], F32, tag="zxp")
                            for k1 in range(3):
                                cnt = KC[k1]
                                nc.tensor.matmul(
                                    out=zxp[:],
                                    lhsT=wxt_c[k1][0:cnt,
                                                   512 * g + 128 * c:
                                                   512 * g + 128 * c + 128],
                                    rhs=xet_c[k1][0:cnt,
                                                  512 * nh:512 * nh + 512],
                                    start=(k1 == 0), stop=(k1 == 2))
                            # zxp cols are (t within half, b); scatter into
                            # zx_all[:, t, g*64+c*16+b]
                            dst = _ap(zx_all[:],
                                      [[192, 32], [1, 16]])
                            dst = bass.AP(
                                tensor=dst.tensor,
                                offset=dst.offset + (nh * 32) * 192
                                + g * 64 + c * 16,
                                ap=dst.ap)
                            nc.vector.tensor_copy(
                                out=dst,
                                in_=zxp[:].rearrange("p (t b) -> p t b",
                                                     b=16))
                    # (pool ctx exit frees wxt/xet SBUF and phase-1 PSUM)

            # ---------------- phase 2: recurrence ----------------
            with (
                tc.tile_pool(name="sp", bufs=1) as sp,
                tc.tile_pool(name="sm", bufs=2) as sm,
                tc.tile_pool(name="wp", bufs=3) as wp,
                tc.tile_pool(name="mp", bufs=2, space="PSUM") as mp,
                tc.tile_pool(name="zp", bufs=1, space="PSUM") as zp,
                tc.tile_pool(name="gp", bufs=1, space="PSUM") as gp,
            ):
                for t in [t_ for _ in range(repeat) for t_ in range(S)]:
                    zxs = zx_all[:, t, :]  # [128, 192] view
                    gt = sm.tile([128, 192], F32, tag="gt")
                    if t > 0:
                        n = 16 * t
                        wrow = wp.tile([1, S * BC], F32, tag="wrow")
                        nc.gpsimd.dma_start(out=wrow[:, 0:n],
                                            in_=wmrow_d[t, 0:n])
                        mpsum = mp.tile([128, 1024], F32, tag="mask")
                        nc.tensor.matmul(
                            out=mpsum[:, 0:min(n, 512)],
                            lhsT=ones_s[0:1, :],
                            rhs=wrow[:, 0:min(n, 512)],
                            start=True, stop=True)
                        if n > 512:
                            nc.tensor.matmul(
                                out=mpsum[:, 512:n],
                                lhsT=ones_s[0:1, :],
                                rhs=wrow[:, 512:n],
                                start=True, stop=True)
                        m_ap = mpsum[:]
                        tb = t - 1  # newest history column (tail)

                        # ---- bulk passes over tau < tb (ready early;
                        # overlap the previous step's tail chain) ----
                        s1 = sp.tile([128, S * 64], F32, tag="s1", bufs=2)
                        s2 = sp.tile([128, S * 64], F32, tag="s2")
                        if tb > 0:
                            nc.vector.tensor_add(
                                out=_ap(s1[:], [[64, tb], [1, 64]]),
                                in0=g_hist[:, 0:tb, :],
                                in1=_bcast_mid(zxs[:, 64:128], tb))
                            nc.scalar.activation(
                                out=s1[:, 0:64 * tb], in_=s1[:, 0:64 * tb],
                                func=AF.Sigmoid)
                            nc.vector.tensor_mul(
                                out=_ap(s2[:], [[64, tb], [16, 4], [1, 16]]),
                                in0=c_hist[:, 0:tb, :].rearrange(
                                    "p t (c b) -> p t c b", b=16),
                                in1=_ap(m_ap, [[16, tb], [0, 4], [1, 16]]))
                            nc.gpsimd.tensor_mul(
                                out=s2[:, 0:64 * tb],
                                in0=s1[:, 0:64 * tb], in1=s2[:, 0:64 * tb])

                        # ---- tail at tau = tb (depends on prev step end) --
                        tsl = slice(64 * tb, 64 * t)
                        m4t = bass.AP(tensor=m_ap.tensor,
                                      offset=m_ap.offset + 16 * tb,
                                      ap=[m_ap.ap[0], [0, 4], [1, 16]])
                        nc.vector.tensor_add(out=s1[:, tsl],
                                             in0=g_hist[:, tb, :],
                                             in1=zxs[:, 64:128])
                        nc.scalar.activation(out=s1[:, tsl], in_=s1[:, tsl],
                                             func=AF.Sigmoid)
                        nc.vector.tensor_mul(
                            out=_ap(s2[:, tsl], [[16, 4], [1, 16]]),
                            in0=c_hist[:, tb, :].rearrange(
                                "p (c b) -> p c b", b=16),
                            in1=m4t)
                        nc.vector.tensor_mul(out=s2[:, tsl], in0=s1[:, tsl],
                                             in1=s2[:, tsl])

                        # fc reduce over tau (bulk) + tail add
                        fct = sm.tile([128, 64], F32, tag="fct")
                        if tb > 0:
                            nc.vector.tensor_reduce(
                                out=fct[:],
                                in_=_ap(s2[:], [[1, 64], [64, tb]]),
                                axis=mybir.AxisListType.X, op=ALU.add)
                            nc.vector.tensor_add(out=fct[:], in0=fct[:],
                                                 in1=s2[:, tsl])
                        else:
                            nc.vector.tensor_copy(out=fct[:], in_=s2[:, tsl])

                        # H^T masked sum (s1 reused as scratch): bulk + tail
                        if tb > 0:
                            nc.vector.tensor_mul(
                                out=_ap(s1[:], [[64, tb], [16, 4], [1, 16]]),
                                in0=ht[:, 0:tb, :].rearrange(
                                    "p t (c b) -> p t c b", b=16),
                                in1=_ap(m_ap, [[16, tb], [0, 4], [1, 16]]))
                        nc.vector.tensor_mul(
                            out=_ap(s1[:, tsl], [[16, 4], [1, 16]]),
                            in0=ht[:, tb, :].rearrange(
                                "p (c b) -> p c b", b=16),
                            in1=m4t)
                        hst = sm.tile([128, 64], F32, tag="hst")
                        if tb > 0:
                            nc.vector.tensor_reduce(
                                out=hst[:],
                                in_=_ap(s1[:], [[1, 64], [64, tb]]),
                                axis=mybir.AxisListType.X, op=ALU.add)
                            nc.vector.tensor_add(out=hst[:], in0=hst[:],
                                                 in1=s1[:, tsl])
                        else:
                            nc.vector.tensor_copy(out=hst[:], in_=s1[:, tsl])

                        # z gates: h-side matmuls, accumulate over k-chunks
                        zpsum = zp.tile([128, 192], F32, tag="z")
                        for g in (0, 2, 1):
                            for c in range(4):
                                for k1 in range(4):
                                    nc.tensor.matmul(
                                        out=zpsum[:, 64 * g + 16 * c:
                                                  64 * g + 16 * c + 16],
                                        lhsT=wht_s[:, k1,
                                                   512 * g + 128 * c:
                                                   512 * g + 128 * c + 128],
                                        rhs=hst[:, 16 * k1:16 * k1 + 16],
                                        start=(k1 == 0), stop=(k1 == 3))
                        nc.vector.tensor_add(out=gt[:], in0=zpsum[:],
                                             in1=zxs)
                    else:
                        nc.vector.tensor_copy(out=gt[:], in_=zxs)

                    # gates -> c, h
                    ga = sm.tile([128, 192], F32, tag="ga")
                    nc.scalar.activation(out=ga[:, 0:128], in_=gt[:, 0:128],
                                         func=AF.Sigmoid)
                    nc.scalar.activation(out=ga[:, 128:192],
                                         in_=gt[:, 128:192], func=AF.Tanh)
                    ctmp = sm.tile([128, 64], F32, tag="ctmp")
                    nc.vector.tensor_mul(out=ctmp[:], in0=ga[:, 0:64],
                                         in1=ga[:, 128:192])
                    if t > 0:
                        nc.vector.tensor_add(out=c_hist[:, t, :],
                                             in0=ctmp[:], in1=fct[:])
                    else:
                        nc.vector.tensor_copy(out=c_hist[:, t, :],
                                              in_=ctmp[:])
                    tct = sm.tile([128, 64], F32, tag="tct")
                    nc.scalar.activation(out=tct[:], in_=c_hist[:, t, :],
                                         func=AF.Tanh)
                    nc.vector.tensor_mul(out=ht[:, t, :], in0=ga[:, 64:128],
                                         in1=tct[:])

                    # G column: g_t = h_t @ Wfh.T
                    if t < S - 1:
                        gpsum = gp.tile([128, 64], F32, tag="g", bufs=2)
                        for m in range(4):
                            for k1 in range(4):
                                nc.tensor.matmul(
                                    out=gpsum[:, 16 * m:16 * m + 16],
                                    lhsT=wgt_s[:, k1, 128 * m:128 * m + 128],
                                    rhs=ht[:, t, 16 * k1:16 * k1 + 16],
                                    start=(k1 == 0), stop=(k1 == 3))
                        nc.vector.tensor_copy(out=g_hist[:, t, :],
                                              in_=gpsum[:])

                # ---------------- output head ----------------
                opsum = gp.tile([BC, L], F32, tag="o")
                for k1 in range(4):
                    nc.tensor.matmul(
                        out=opsum[:],
                        lhsT=ht[:, S - 1, 16 * k1:16 * k1 + 16],
                        rhs=wout_s[:, k1, :],
                        start=(k1 == 0), stop=(k1 == 3))
                osb = sm.tile([BC, L], F32, tag="osb")
                nc.vector.tensor_add(out=osb[:], in0=opsum[:], in1=bout_s[:])
                nc.gpsimd.dma_start(out=out_d[:], in_=osb[:])

    nc.finalize()
    _prog_cache[key] = nc
    return nc


def _host_prep(x, bfs, children, embed, Wix, bix, Wih, bih, Wfx, bfx, Wfh,
               bfh, Wux, bux, Wuh, buh, Wout, bout):
    """Build shared weight tensors + per-core xet/wmrow."""
    f32 = np.float32
    wxt = np.zeros((EK, 3 * H), f32)
    for g, (W, bvec) in enumerate([
            (Wix, bix + bih), (Wfx, bfx + bfh), (Wux, bux + buh)]):
        wxt[:E, 512 * g:512 * (g + 1)] = W.T.astype(f32)
        wxt[E, 512 * g:512 * (g + 1)] = bvec.astype(f32)
    wht = np.concatenate(
        [Wih.T, Wfh.T, Wuh.T], axis=1).astype(f32)          # [512, 1536]
    wgt = np.ascontiguousarray(Wfh.T.astype(f32))           # [512, 512]
    wout = np.ascontiguousarray(Wout.T.astype(f32))         # [512, L]
    bout_rep = np.tile(bout.astype(f32)[None, :], (BC, 1))  # [16, L]

    xets, wmrows = [], []
    for c0 in range(NCORES):
        bs = slice(c0 * BC, (c0 + 1) * BC)
        bfs_c = bfs[bs]                      # [16, 64]
        x_c = x[bs]
        ch_c = children[bs]                  # [16, 64, 64]
        tok = np.take_along_axis(x_c, bfs_c, axis=1)   # [16, 64]
        xe = embed[tok]                      # [16, 64, 300]
        xet = np.zeros((EK, S * BC), f32)
        xet[:E] = xe.transpose(2, 1, 0).reshape(E, S * BC)  # col = t*16+b
        xet[E] = 1.0
        xets.append(xet)

        wm = np.zeros((S, S, BC), f32)       # [t, tau, b]
        lastw = -np.ones((BC, S), np.int64)  # node -> last writing step
        barange = np.arange(BC)
        for t in range(S):
            cur = bfs_c[:, t]
            ch_of_cur = ch_c[barange, cur, :]          # [16, 64] node-mask
            for b in range(BC):
                nodes = np.nonzero((lastw[b] >= 0) & (ch_of_cur[b] > 0))[0]
                if nodes.size:
                    wm[t, lastw[b, nodes], b] = 1.0
            lastw[barange, cur] = t
        wmrows.append(np.ascontiguousarray(wm.reshape(S, S * BC)))

    return wxt, wht, wgt, wout, bout_rep, xets, wmrows


def _get_runner(repeat=1):
    """Build (once) a cached sharded jax callable around the Bass program."""
    rkey = ("runner", repeat)
    if rkey in _prog_cache:
        return _prog_cache[rkey]
    import jax
    from jax.experimental.shard_map import shard_map
    from jax.sharding import Mesh, PartitionSpec
    from concourse import bass2jax

    nc = _build_program(repeat)
    bass2jax.install_neuronx_cc_hook()
    pname = nc.partition_id_tensor.name if nc.partition_id_tensor else None
    in_names, out_names, out_avals, out_shapes, out_dtypes = [], [], [], [], []
    for alloc in nc.m.functions[0].allocations:
        if not isinstance(alloc, mybir.MemoryLocationSet):
            continue
        name = alloc.memorylocations[0].name
        if alloc.kind == "ExternalInput":
            if name != pname:
                in_names.append(name)
        elif alloc.kind == "ExternalOutput":
            out_names.append(name)
            shape = tuple(alloc.tensor_shape)
            dtype = mybir.dt.np(alloc.dtype)
            out_avals.append(jax.core.ShapedArray(shape, dtype))
            out_shapes.append(shape)
            out_dtypes.append(dtype)
    n_params = len(in_names)
    all_in_names = list(in_names) + list(out_names)
    if pname is not None:
        all_in_names.append(pname)
    donate = tuple(range(n_params, n_params + len(out_names)))

    def _body(*args):
        operands = list(args)
        if pname is not None:
            operands.append(bass2jax.partition_id_tensor())
        outs = bass2jax._bass_exec_p.bind(
            *operands,
            out_avals=tuple(out_avals),
            in_names=tuple(all_in_names),
            out_names=tuple(out_names),
            lowering_input_output_aliases=(),
            sim_require_finite=True,
            sim_require_nnan=True,
            nc=nc,
        )
        return tuple(outs)

    devices = jax.devices()[:NCORES]
    mesh = Mesh(np.asarray(devices), ("core",))
    in_specs = (PartitionSpec("core"),) * (n_params + len(out_names))
    out_specs = (PartitionSpec("core"),) * len(out_names)
    sharded = jax.jit(
        shard_map(_body, mesh=mesh, in_specs=in_specs, out_specs=out_specs,
                  check_rep=False),
        donate_argnums=donate, keep_unused=True)
    runner = (sharded, in_names, out_names, out_shapes, out_dtypes)
    _prog_cache[rkey] = runner
    return runner


LAST_EXEC_S = None


def kernel(**inputs):
    global LAST_RESULT, LAST_EXEC_S
    import time

    x = np.asarray(inputs["x"]).astype(np.int64)
    bfs = np.asarray(inputs["bfs"]).astype(np.int64)
    children = np.asarray(inputs["children"]).astype(np.int64)
    embed = np.ascontiguousarray(np.asarray(inputs["embed"], dtype=np.float32))
    wargs = {k: np.asarray(inputs[k], dtype=np.float32)
             for k in ["Wix", "bix", "Wih", "bih", "Wfx", "bfx", "Wfh", "bfh",
                       "Wux", "bux", "Wuh", "buh", "Wout", "bout"]}

    wxt, wht, wgt, wout, bout_rep, xets, wmrows = _host_prep(
        x, bfs, children, embed, **wargs)

    sharded, in_names, out_names, out_shapes, out_dtypes = _get_runner()
    data = dict(xet=xets, wxt=[wxt] * NCORES, wht=[wht] * NCORES,
                wgt=[wgt] * NCORES, wout=[wout] * NCORES,
                bout_rep=[bout_rep] * NCORES, wmrow=wmrows)
    concat_in = [np.concatenate(data[nm], axis=0) for nm in in_names]
    zero_outs = [np.zeros((NCORES * sh[0],) + sh[1:], dt)
                 for sh, dt in zip(out_shapes, out_dtypes)]

    t0 = time.perf_counter()
    try:
        outs = sharded(*concat_in, *zero_outs)
        outs = [np.asarray(o) for o in outs]
    except Exception:
        # transient NRT/axon failures: retry once with fresh output buffers
        zero_outs = [np.zeros((NCORES * sh[0],) + sh[1:], dt)
                     for sh, dt in zip(out_shapes, out_dtypes)]
        outs = sharded(*concat_in, *zero_outs)
        outs = [np.asarray(o) for o in outs]
    t1 = time.perf_counter()
    LAST_EXEC_S = t1 - t0

    out = outs[out_names.index("out")]  # [B, L] already concat over cores
    return np.ascontiguousarray(out.astype(np.float32))

